# revision 1
# baseline (speedup 1.0000x reference)
"""BLOOM attention layer on 8 Trainium2 NeuronCores.

Sharding: tensor-parallel over heads (4 heads/core) x data-parallel over batch
(B=2), mesh [DP=2, TP=4].  Core c handles batch b=c//4, heads 4*(c%4)..4*(c%4)+3.

Per-core device kernel (all matmuls bf16 operands, f32 PSUM accumulation):
  1. QKV projection from pre-transposed activations Xt [H,S]:
       Qt,Kt computed transposed [d, S] (d on partitions), V natural [S, d].
       inv_norm folded into Wq host-side; biases via ACT-Identity / K=1 matmul.
  2. Per head, per 512-wide query slice: st[k,q] = Kt_tile^T-free matmul,
       E = exp(st + alibi_k) (no max subtraction -- scores are O(1) bounded),
       causal block skipping + boundary mask multiply,
       ctx^T[d,q] += V_tile^T @ E and sums[q] += ones @ E accumulated in PSUM,
       ctx^T *= 1/sums broadcast across partitions.
  3. Dense partial: out_part[s,h'] = sum_c ctx^T[c,s]^T @ Wd[c,h'] -> f32 DRAM.
Host: shard/pre-transpose/cast inputs, then sum the 4 TP partials per batch and
add b_dense + residual.
"""

import numpy as np
import ml_dtypes

bf16 = ml_dtypes.bfloat16

B, S, H, NH = 2, 2048, 2048, 16
HD = H // NH  # 128
INV_NORM = 1.0 / float(np.sqrt(HD))
NCORES = 8
TP = 4
HPC = NH // TP  # heads per core = 4
QSL = 512      # query slice width
KTL = 128      # key tile length
N_QS = S // QSL   # 4
N_KT = S // KTL   # 16
N_HT = H // 128   # 16 contraction tiles for QKV proj

# block kinds
SKIP, FULL, PARTIAL = 0, 1, 2

_program_cache: dict = {}


def _analyze_mask(mask2d: np.ndarray):
    """mask2d [S,S] bool, True = disallowed.  Returns block kinds [N_QS][N_KT],
    pattern index per block, and deduped allow-patterns [128, npat*512] layout list."""
    kinds = [[SKIP] * N_KT for _ in range(N_QS)]
    pidx = [[-1] * N_KT for _ in range(N_QS)]
    patterns = []
    pat_lookup = {}
    for qs in range(N_QS):
        for kt in range(N_KT):
            allow = ~mask2d[qs * QSL:(qs + 1) * QSL, kt * KTL:(kt + 1) * KTL]  # [512q,128k]
            if not allow.any():
                kinds[qs][kt] = SKIP
            elif allow.all():
                kinds[qs][kt] = FULL
            else:
                kinds[qs][kt] = PARTIAL
                pat = np.ascontiguousarray(allow.T)  # [128k, 512q]
                key = pat.tobytes()
                if key not in pat_lookup:
                    pat_lookup[key] = len(patterns)
                    patterns.append(pat)
                pidx[qs][kt] = pat_lookup[key]
    return kinds, pidx, patterns


def _build_program(kinds, pidx, npat):
    import concourse.tile as tile
    import concourse.mybir as mybir
    from concourse import bacc

    f32 = mybir.dt.float32
    bf = mybir.dt.bfloat16
    AFT = mybir.ActivationFunctionType

    nc = bacc.Bacc(
        "TRN2",
        target_bir_lowering=False,
        debug=False,
        enable_asserts=False,
        num_devices=NCORES,
    )
    xt_d = nc.dram_tensor("xt", [H, S], bf, kind="ExternalInput")
    wqk_d = nc.dram_tensor("wqk", [H, 2 * HPC * 128], bf, kind="ExternalInput")
    wv_d = nc.dram_tensor("wv", [H, HPC * 128], bf, kind="ExternalInput")
    wd_d = nc.dram_tensor("wd", [HPC * 128, H], bf, kind="ExternalInput")
    bqk_d = nc.dram_tensor("bqk", [128, 2 * HPC], f32, kind="ExternalInput")
    bv_d = nc.dram_tensor("bv", [1, HPC * 128], bf, kind="ExternalInput")
    alibi_d = nc.dram_tensor("alibi", [128, HPC * N_KT], f32, kind="ExternalInput")
    if npat > 0:
        maskpat_d = nc.dram_tensor("maskpat", [128, npat, QSL], bf, kind="ExternalInput")
    out_d = nc.dram_tensor("out", [S, H], f32, kind="ExternalOutput")

    xt_r = xt_d.rearrange("(ho p) s -> p ho s", p=128)        # [128,16,2048]
    wqk_r = wqk_d.rearrange("(ho p) c -> p ho c", p=128)      # [128,16,1024]
    wv_r = wv_d.rearrange("(ho p) c -> p ho c", p=128)        # [128,16,512]
    wd_r = wd_d.rearrange("(co p) h -> p co h", p=128)        # [128,4,2048]
    out_r = out_d.rearrange("(so p) h -> p so h", p=128)      # [128,16,2048]

    NCI = 2 * HPC  # 8 qk column tiles

    with tile.TileContext(nc) as tc:
        with (
            tc.tile_pool(name="singles", bufs=1) as singles,
            tc.tile_pool(name="stream", bufs=3) as stream,      # wqk streaming
            tc.tile_pool(name="epool", bufs=8) as epool,        # E tiles
            tc.tile_pool(name="rpool", bufs=2) as rpool,        # recipb
            tc.tile_pool(name="outstage", bufs=4) as outstage,
        ):
            # ---- resident tiles; DMA issue order tracks first use ----
            xt_sb = singles.tile([128, N_HT, S], bf, tag="xt_sb", name="xt_sb")
            wtile0 = stream.tile([128, N_HT, 128], bf, tag="wstream", name="wqk_a_0")
            nc.sync.dma_start(out=wtile0, in_=wqk_r[:, :, 0:128])
            # s-slice 0 of Xt in two ht-chunks so the first matmuls start early
            for hc in range(2):
                nc.sync.dma_start(
                    out=xt_sb[:, hc * 8:(hc + 1) * 8, 0:QSL],
                    in_=xt_r[:, hc * 8:(hc + 1) * 8, 0:QSL],
                )
            bqk_sb = singles.tile([128, NCI], f32, tag="bqk_sb", name="bqk_sb")
            nc.sync.dma_start(out=bqk_sb, in_=bqk_d[:])
            bv_sb = singles.tile([1, HPC * 128], bf, tag="bv_sb", name="bv_sb")
            nc.sync.dma_start(out=bv_sb, in_=bv_d[:])
            ones_col = singles.tile([1, 128], bf, tag="ones_col", name="ones_col")
            nc.vector.memset(ones_col, 1.0)
            ones_mat = singles.tile([128, 128], bf, tag="ones_mat", name="ones_mat")
            nc.vector.memset(ones_mat, 1.0)

            qkt_sb = singles.tile([128, NCI, S], bf, tag="qkt_sb", name="qkt_sb")   # Qt/Kt per head
            v_sb = singles.tile([128, N_KT, HPC * 128], bf, tag="v_sb", name="v_sb")
            ctx_sb = singles.tile([128, HPC, S], bf, tag="ctx_sb", name="ctx_sb")

            # ================= phase 1: QKV projection =================
            with tc.tile_pool(name="ps_p1", bufs=8, space="PSUM") as ps_p1:
                # pass 1: Qt/Kt for s-slice 0 (starts after ~1MB of DMA)
                for ci in range(NCI):
                    if ci == 0:
                        wtile = wtile0
                    else:
                        wtile = stream.tile([128, N_HT, 128], bf, tag="wstream", name=f"wqk_a_{ci}")
                        nc.sync.dma_start(out=wtile, in_=wqk_r[:, :, ci * 128:(ci + 1) * 128])
                    ps0 = ps_p1.tile([128, QSL], f32, tag="psp1", name=f"qk0_{ci}")
                    for ht in range(N_HT):
                        nc.tensor.matmul(
                            ps0,
                            lhsT=wtile[:, ht, :],
                            rhs=xt_sb[:, ht, 0:QSL],
                            start=(ht == 0),
                            stop=(ht == N_HT - 1),
                        )
                    nc.scalar.activation(
                        out=qkt_sb[:, ci, 0:QSL],
                        in_=ps0,
                        func=AFT.Identity,
                        bias=bqk_sb[:, ci:ci + 1],
                        scale=1.0,
                    )

                # next activation slice first, then prefetch the first two
                # pass-2 weight tiles, then the rest of the bulk loads
                nc.sync.dma_start(
                    out=xt_sb[:, :, QSL:2 * QSL], in_=xt_r[:, :, QSL:2 * QSL]
                )
                wtiles_b = {}
                for ci in range(2):
                    wt = stream.tile([128, N_HT, 128], bf, tag="wstream", name=f"wqk_b_{ci}")
                    nc.sync.dma_start(out=wt, in_=wqk_r[:, :, ci * 128:(ci + 1) * 128])
                    wtiles_b[ci] = wt
                for ss in range(2, N_QS):
                    nc.sync.dma_start(
                        out=xt_sb[:, :, ss * QSL:(ss + 1) * QSL],
                        in_=xt_r[:, :, ss * QSL:(ss + 1) * QSL],
                    )
                wv_sb = singles.tile([128, N_HT, HPC * 128], bf, tag="wv_sb", name="wv_sb")
                nc.sync.dma_start(out=wv_sb, in_=wv_r)

                # pass 2: Qt/Kt for s-slices 1..3
                for ci in range(NCI):
                    if ci in wtiles_b:
                        wtile = wtiles_b[ci]
                    else:
                        wtile = stream.tile([128, N_HT, 128], bf, tag="wstream", name=f"wqk_b_{ci}")
                        nc.sync.dma_start(out=wtile, in_=wqk_r[:, :, ci * 128:(ci + 1) * 128])
                    pss = [
                        ps_p1.tile([128, QSL], f32, tag="psp1", name=f"qk_{ci}_{ss}")
                        for ss in range(1, N_QS)
                    ]
                    for ht in range(N_HT):
                        for j, ss in enumerate(range(1, N_QS)):
                            nc.tensor.matmul(
                                pss[j],
                                lhsT=wtile[:, ht, :],
                                rhs=xt_sb[:, ht, ss * QSL:(ss + 1) * QSL],
                                start=(ht == 0),
                                stop=(ht == N_HT - 1),
                            )
                    for j, ss in enumerate(range(1, N_QS)):
                        nc.scalar.activation(
                            out=qkt_sb[:, ci, ss * QSL:(ss + 1) * QSL],
                            in_=pss[j],
                            func=AFT.Identity,
                            bias=bqk_sb[:, ci:ci + 1],
                            scale=1.0,
                        )

                # loads for phase 1b / 2
                wv_sb = singles.tile([128, N_HT, HPC * 128], bf, tag="wv_sb", name="wv_sb")
                nc.sync.dma_start(out=wv_sb, in_=wv_r)
                alibi_sb = singles.tile([128, HPC * N_KT], f32, tag="alibi_sb", name="alibi_sb")
                nc.sync.dma_start(out=alibi_sb, in_=alibi_d[:])
                if npat > 0:
                    mask_sb = singles.tile([128, npat, QSL], bf, tag="mask_sb", name="mask_sb")
                    nc.sync.dma_start(out=mask_sb, in_=maskpat_d[:])

                # phase 1b: V
                for sg in range(4):
                    psv = [
                        ps_p1.tile([128, HPC * 128], f32, tag="psp1", name=f"v_{sg}_{j}")
                        for j in range(4)
                    ]
                    for ht in range(N_HT):
                        for j in range(4):
                            sti = sg * 4 + j
                            nc.tensor.matmul(
                                psv[j],
                                lhsT=xt_sb[:, ht, sti * 128:(sti + 1) * 128],
                                rhs=wv_sb[:, ht, :],
                                start=(ht == 0),
                                stop=False,
                            )
                    for j in range(4):
                        sti = sg * 4 + j
                        nc.tensor.matmul(
                            psv[j], lhsT=ones_col, rhs=bv_sb, start=False, stop=True
                        )
                        nc.vector.tensor_copy(out=v_sb[:, sti, :], in_=psv[j])

            # ================= phase 2: attention =================
            with (
                tc.tile_pool(name="ps_st", bufs=4, space="PSUM") as ps_st,
                tc.tile_pool(name="ps_ctx", bufs=2, space="PSUM") as ps_ctx,
                tc.tile_pool(name="ps_sums", bufs=2, space="PSUM") as ps_sums,
            ):
                for h in range(HPC):
                    q_ci, k_ci = 2 * h, 2 * h + 1
                    for qs in range(N_QS - 1, -1, -1):
                        kts = [kt for kt in range(N_KT) if kinds[qs][kt] != SKIP]
                        ctx_ps = ps_ctx.tile([128, QSL], f32, tag="ctxps", name=f"ctx_{h}_{qs}")
                        sums_ps = ps_sums.tile([128, QSL], f32, tag="sumsps", name=f"sums_{h}_{qs}")
                        for i, kt in enumerate(kts):
                            st_ps = ps_st.tile([128, QSL], f32, tag="stps", name=f"st_{h}_{qs}_{kt}")
                            nc.tensor.matmul(
                                st_ps,
                                lhsT=qkt_sb[:, k_ci, kt * KTL:(kt + 1) * KTL],
                                rhs=qkt_sb[:, q_ci, qs * QSL:(qs + 1) * QSL],
                                start=True,
                                stop=True,
                            )
                            e_t = epool.tile([128, QSL], bf, tag="etile", name=f"e_{h}_{qs}_{kt}")
                            nc.scalar.activation(
                                out=e_t,
                                in_=st_ps,
                                func=AFT.Exp,
                                bias=alibi_sb[:, h * N_KT + kt: h * N_KT + kt + 1],
                                scale=1.0,
                            )
                            if kinds[qs][kt] == PARTIAL:
                                nc.vector.tensor_mul(
                                    out=e_t, in0=e_t, in1=mask_sb[:, pidx[qs][kt], :]
                                )
                            first, last = (i == 0), (i == len(kts) - 1)
                            nc.tensor.matmul(
                                ctx_ps,
                                lhsT=v_sb[:, kt, h * 128:(h + 1) * 128],
                                rhs=e_t,
                                start=first,
                                stop=last,
                            )
                            nc.tensor.matmul(
                                sums_ps, lhsT=ones_mat, rhs=e_t, start=first, stop=last
                            )
                        recipb = rpool.tile([128, QSL], f32, tag="recipb", name=f"recipb_{h}_{qs}")
                        nc.vector.reciprocal(out=recipb, in_=sums_ps)
                        nc.vector.tensor_mul(
                            out=ctx_sb[:, h, qs * QSL:(qs + 1) * QSL],
                            in0=ctx_ps,
                            in1=recipb,
                        )

            # ================= phase 3: dense partial =================
            wd_sb = singles.tile([128, HPC, H], bf, tag="wd_sb", name="wd_sb")
            nc.sync.dma_start(out=wd_sb, in_=wd_r)
            with tc.tile_pool(name="ps_d", bufs=8, space="PSUM") as ps_d:
                for so in range(N_KT):  # 16 s tiles
                    psd = [
                        ps_d.tile([128, QSL], f32, tag="psd", name=f"d_{so}_{hs}")
                        for hs in range(4)
                    ]
                    for ct in range(HPC):
                        for hs in range(4):
                            nc.tensor.matmul(
                                psd[hs],
                                lhsT=ctx_sb[:, ct, so * 128:(so + 1) * 128],
                                rhs=wd_sb[:, ct, hs * QSL:(hs + 1) * QSL],
                                start=(ct == 0),
                                stop=(ct == HPC - 1),
                            )
                    for hs in range(4):
                        ot = outstage.tile([128, QSL], f32, tag="ostage", name=f"o_{so}_{hs}")
                        nc.vector.tensor_copy(out=ot, in_=psd[hs])
                        nc.sync.dma_start(
                            out=out_r[:, so, hs * QSL:(hs + 1) * QSL], in_=ot
                        )

    nc.compile()
    return nc


def _prepare_core_inputs(inputs):
    hs = np.asarray(inputs["hidden_states"], np.float32)
    alibi = np.asarray(inputs["alibi"], np.float32).reshape(B, NH, S)
    mask = np.asarray(inputs["attention_mask"], bool)
    W_qkv = np.asarray(inputs["W_qkv"], np.float32).reshape(H, NH, 3, HD)
    b_qkv = np.asarray(inputs["b_qkv"], np.float32).reshape(NH, 3, HD)
    W_dense = np.asarray(inputs["W_dense"], np.float32)

    # masks must agree across batch (broadcast in the reference setup)
    for b in range(1, B):
        assert np.array_equal(mask[0, 0], mask[b, 0]), "per-batch masks differ"
    kinds, pidx, patterns = _analyze_mask(mask[0, 0])
    npat = len(patterns)

    xt = [np.ascontiguousarray(hs[b].T).astype(bf16) for b in range(B)]
    if npat > 0:
        maskpat = np.stack(patterns, axis=1).astype(bf16)  # [128, npat, 512]

    in_maps = []
    for c in range(NCORES):
        b, g = divmod(c, TP)
        heads = range(HPC * g, HPC * g + HPC)
        wqk = np.empty((H, 2 * HPC * 128), np.float32)
        bqk = np.empty((2 * HPC, 128), np.float32)
        wv = np.empty((H, HPC * 128), np.float32)
        bv = np.empty((1, HPC * 128), np.float32)
        wd = np.empty((HPC * 128, H), np.float32)
        al = np.empty((128, HPC * N_KT), np.float32)
        for i, hh in enumerate(heads):
            wqk[:, (2 * i) * 128:(2 * i + 1) * 128] = W_qkv[:, hh, 0, :] * INV_NORM
            wqk[:, (2 * i + 1) * 128:(2 * i + 2) * 128] = W_qkv[:, hh, 1, :]
            bqk[2 * i] = b_qkv[hh, 0, :] * INV_NORM
            bqk[2 * i + 1] = b_qkv[hh, 1, :]
            wv[:, i * 128:(i + 1) * 128] = W_qkv[:, hh, 2, :]
            bv[0, i * 128:(i + 1) * 128] = b_qkv[hh, 2, :]
            wd[i * 128:(i + 1) * 128, :] = W_dense[hh * HD:(hh + 1) * HD, :]
            al[:, i * N_KT:(i + 1) * N_KT] = alibi[b, hh].reshape(N_KT, 128).T
        m = {
            "xt": xt[b],
            "wqk": wqk.astype(bf16),
            "wv": wv.astype(bf16),
            "wd": wd.astype(bf16),
            "bqk": np.ascontiguousarray(bqk.T),
            "bv": bv.astype(bf16),
            "alibi": al,
        }
        if npat > 0:
            m["maskpat"] = maskpat
        in_maps.append(m)
    return in_maps, kinds, pidx, npat


def _run(inputs, trace=False, trace_cores=None):
    from concourse.bass_utils import run_bass_kernel_spmd

    in_maps, kinds, pidx, npat = _prepare_core_inputs(inputs)
    key = (tuple(tuple(r) for r in kinds), tuple(tuple(r) for r in pidx), npat)
    if key not in _program_cache:
        _program_cache[key] = _build_program(kinds, pidx, npat)
    nc = _program_cache[key]
    res = run_bass_kernel_spmd(
        nc,
        in_maps,
        core_ids=list(range(NCORES)),
        trace=trace,
        trace_cores=trace_cores,
    )

    residual = np.asarray(inputs["residual"], np.float32)
    b_dense = np.asarray(inputs["b_dense"], np.float32)
    out = np.empty((B, S, H), np.float32)
    for b in range(B):
        acc = res.results[b * TP + 0]["out"].copy()
        for g in range(1, TP):
            acc += res.results[b * TP + g]["out"]
        out[b] = acc + b_dense[None, :] + residual[b]
    return out, res


def kernel(**inputs) -> np.ndarray:
    out, _ = _run(inputs, trace=False)
    return out

